# revision 29
# baseline (speedup 1.0000x reference)
"""Block-diagonal MLP kernel for Trainium2 (8 NeuronCores, expert-sharded).

Computes out = blockdiag_matmul(x, weights) + bias where
  x: [4, 2048, 4096] f32, weights: [32, 128, 128] f32, bias: [4096] f32.

Strategy: shard the 32 diagonal blocks across 8 cores (4 blocks = 512
feature columns each); every core sees all 8192 flattened rows of its
512-column slice.  Per-core DMA is 16.78 MB in + 16.78 MB out + 0.2 MB
consts.  Loads and stores co-flow on the two HWDGE rings, which
together sustain ~430 GB/s (the SBUF-AXI fabric ceiling) -- the body
floor is ~79 us, so the schedule's whole job is to avoid solo-load /
solo-store phases that cap at ~240-340 GB/s.

The host packs each core's x shard as [128, 32768] (partition p holds
the rows congruent to p mod 128, 64 row-groups side by side), so DMA
per-partition lines are 8 KiB (2 KiB descriptors measured ~35% slower
per ring).  x streams through a rotating pool of [128, 2048] chunk
buffers (4 groups each): each load waits for the transposes of the
chunk 4 buffers back, so loads self-pace to compute rate instead of
front-loading, and stores (ready from ~14 us) overlap loads for the
whole body.  Loads ride the ACT ring, stores the Sync ring; the last
two out-tiles store per-group alternating across both rings so the
tail drains at both-ring rate.

Per 512-column group: PE transpose-mode matmuls (fp32) put the
contraction dim on partitions; ACT evacuates the transpose to SBUF
casting fp32->bf16 (free cast -- bf16 halves the real matmul cost);
bf16 matmuls against SBUF-resident bf16 weights (host-cast, the same
4 blocks for all 64 groups); DVE evacuates with the bias add fused.
Transposes run two groups ahead of the consuming matmuls.  The bias
[1,512] row is broadcast to 128 partitions once on-chip via a K=1
ones-matmul.  bf16 is only used for matmul operands (fp32 PSUM
accumulation); max rel err vs the fp32 reference ~2e-3 (gate 2e-2).
"""
import numpy as np
import ml_dtypes
from contextlib import ExitStack

import concourse.mybir as mybir
import concourse.tile as tile
from concourse import bacc
from concourse.bass_utils import run_bass_kernel_spmd

F32 = mybir.dt.float32
BF16 = mybir.dt.bfloat16

SIZE = 4096
NB = 32            # number of diagonal blocks
BLK = 128          # block size
N_CORES = 8
NB_CORE = NB // N_CORES        # 4 blocks per core
C_CORE = NB_CORE * BLK         # 512 feature columns per core
B_FULL = 4 * 2048              # 8192 flattened rows (all on every core)
GROUPS = B_FULL // 128         # 64 row-groups of [128, 512]
XP_COLS = GROUPS * C_CORE      # 32768 packed columns
G_PER_CHUNK = 8                # groups per load chunk [128, 4096]
N_CHUNKS = GROUPS // G_PER_CHUNK
G_PER_OUT = 4                  # groups per store tile [128, 2048]
TAIL_GROUPS = 8                # last groups stored per-group on both rings

_NC_CACHE = {}


def _build_nc():
    nc = bacc.Bacc()
    x_d = nc.declare_dram_parameter("x", [128, XP_COLS], F32, isOutput=False)
    w_d = nc.declare_dram_parameter("weights", [BLK, C_CORE], BF16, isOutput=False)
    b_d = nc.declare_dram_parameter("bias", [1, C_CORE], F32, isOutput=False)
    i_d = nc.declare_dram_parameter("ident", [BLK, BLK], BF16, isOutput=False)
    if32_d = nc.declare_dram_parameter("ident32", [BLK, BLK], F32, isOutput=False)
    n_d = nc.declare_dram_parameter("ones", [1, BLK], F32, isOutput=False)
    o_d = nc.declare_dram_parameter("out", [128, XP_COLS], F32, isOutput=True)

    with tile.TileContext(nc) as tc, ExitStack() as ctx:
        consts = ctx.enter_context(tc.tile_pool(name="consts", bufs=1))
        x_pool = ctx.enter_context(tc.tile_pool(name="x", bufs=3))
        xf_pool = ctx.enter_context(tc.tile_pool(name="xf", bufs=2))
        xt_pool = ctx.enter_context(tc.tile_pool(name="xt", bufs=4))
        out_pool = ctx.enter_context(tc.tile_pool(name="out", bufs=4))
        tp_pool = ctx.enter_context(tc.tile_pool(name="tp", bufs=3, space="PSUM"))
        mp_pool = ctx.enter_context(tc.tile_pool(name="mp", bufs=3, space="PSUM"))
        bp_pool = ctx.enter_context(tc.tile_pool(name="bp", bufs=1, space="PSUM"))

        ident = consts.tile([BLK, BLK], BF16)
        ident32 = consts.tile([BLK, BLK], F32)
        ones = consts.tile([1, BLK], F32)
        w_sb = consts.tile([BLK, C_CORE], BF16)
        b_row = consts.tile([1, C_CORE], F32)
        bias_sb = consts.tile([128, C_CORE], F32)

        # Consts: identity (needed by the first transpose ~10.5 us in)
        # leads the Sync ring; weights/bias lead the ACT ring ahead of
        # the x stream.
        nc.sync.dma_start(out=ident32, in_=if32_d[:, :])
        nc.sync.dma_start(out=ident, in_=i_d[:, :])
        nc.sync.dma_start(out=ones, in_=n_d[:, :])
        nc.scalar.dma_start(out=w_sb, in_=w_d[:, :])
        nc.scalar.dma_start(out=b_row, in_=b_d[:, :])

        # Broadcast bias across partitions: [128,512] = ones.T @ b_row.
        bp = bp_pool.tile([128, C_CORE], F32)
        nc.tensor.matmul(bp, ones, b_row, start=True, stop=True)
        nc.vector.tensor_copy(bias_sb, bp)

        x_chunks = [None] * N_CHUNKS

        def emit_load(c):
            base = c * G_PER_CHUNK * C_CORE
            if c < 2:
                # Head: the HWDGE rings are store-idle for the first
                # ~20 us, so the first two chunks ride them as f32 at
                # full ring rate while SWDGE streams the rest -- HBM
                # saturates from the first byte.
                xc = xf_pool.tile([128, G_PER_CHUNK * C_CORE], F32)
                eng = nc.scalar if c == 0 else nc.sync
                if c == 0:
                    # split so the first transposes start earlier
                    eng.dma_start(out=xc[:, 0:512], in_=x_d[:, 0:512])
                    eng.dma_start(out=xc[:, 512:2048], in_=x_d[:, 512:2048])
                    eng.dma_start(out=xc[:, 2048:4096], in_=x_d[:, 2048:4096])
                else:
                    eng.dma_start(out=xc, in_=x_d[:, base:base + G_PER_CHUNK * C_CORE])
            else:
                # SWDGE (gpsimd) DMA casts f32 DRAM -> bf16 SBUF inline in
                # the DMA engines, freeing both HWDGE rings for stores.
                xc = x_pool.tile([128, G_PER_CHUNK * C_CORE], BF16)
                nc.gpsimd.dma_start(out=xc, in_=x_d[:, base:base + G_PER_CHUNK * C_CORE])
            x_chunks[c] = xc

        for c in range(2):
            emit_load(c)

        def emit_transposes(g):
            xc = x_chunks[g // G_PER_CHUNK]
            f32_chunk = g // G_PER_CHUNK < 2
            tp = tp_pool.tile([128, C_CORE], F32 if f32_chunk else BF16)
            idn = ident32 if f32_chunk else ident
            gb = (g % G_PER_CHUNK) * C_CORE
            for j in range(NB_CORE):
                nc.tensor.matmul(
                    tp[:, j * 128:(j + 1) * 128],
                    xc[:, gb + j * 128:gb + (j + 1) * 128],
                    idn,
                    is_transpose=True,
                    start=(j == 0),
                    stop=(j == NB_CORE - 1),
                )
            xt = xt_pool.tile([128, C_CORE], BF16)
            nc.scalar.copy(xt, tp)   # PSUM -> SBUF bf16 (casts for f32 chunks)
            return xt

        xt_q = [emit_transposes(0), emit_transposes(1)]
        out_tile = None
        for g in range(GROUPS):
            if g % G_PER_OUT == 0:
                out_tile = out_pool.tile([128, G_PER_OUT * C_CORE], F32)
            # prefetch: 2 chunks (16 groups) ahead of the transposes,
            # which themselves run 2 groups ahead of the matmuls here
            if g % G_PER_CHUNK == 0 and (gc := g // G_PER_CHUNK + 2) < N_CHUNKS:
                emit_load(gc)
            xt = xt_q.pop(0)
            if g + 2 < GROUPS:
                xt_q.append(emit_transposes(g + 2))
            mp = mp_pool.tile([128, C_CORE], F32)
            for j in range(NB_CORE):
                nc.tensor.matmul(
                    mp[:, j * 128:(j + 1) * 128],
                    xt[:, j * 128:(j + 1) * 128],
                    w_sb[:, j * 128:(j + 1) * 128],
                    start=(j == 0),
                    stop=(j == NB_CORE - 1),
                )
            gi = (g % G_PER_OUT) * C_CORE
            nc.vector.tensor_add(out_tile[:, gi:gi + C_CORE], mp, bias_sb)
            # Loads ride the SWDGE queue, so BOTH HWDGE rings carry
            # stores: alternate out-tiles between them (8.4 MB each).
            if g >= GROUPS - TAIL_GROUPS:
                # tail: store per-pair alternating rings so the kernel
                # tail only waits on 512 KiB.
                if g % 2 == 1:
                    eng = nc.sync if g % 4 == 1 else nc.scalar
                    cols = slice((g - 1) * C_CORE, (g + 1) * C_CORE)
                    eng.dma_start(
                        out=o_d[:, cols],
                        in_=out_tile[:, gi - C_CORE:gi + C_CORE],
                    )
            elif g % G_PER_OUT == G_PER_OUT - 1:
                t = g // G_PER_OUT
                eng = nc.sync if t % 2 == 0 else nc.scalar
                cols = slice(t * G_PER_OUT * C_CORE, (t + 1) * G_PER_OUT * C_CORE)
                eng.dma_start(out=o_d[:, cols], in_=out_tile)

    nc.compile()
    return nc


def _get_nc():
    if "nc" not in _NC_CACHE:
        _NC_CACHE["nc"] = _build_nc()
    return _NC_CACHE["nc"]


def _run(inputs, trace=False):
    x = np.asarray(inputs["x"], dtype=np.float32)
    weights = np.asarray(inputs["weights"], dtype=np.float32)
    bias = np.asarray(inputs["bias"], dtype=np.float32)
    orig_shape = x.shape
    xf = x.reshape(B_FULL, SIZE)
    ident32 = np.eye(BLK, dtype=np.float32)
    ident = ident32.astype(ml_dtypes.bfloat16)
    ones = np.ones((1, BLK), dtype=np.float32)

    nc = _get_nc()
    in_maps = []
    for i in range(N_CORES):
        cols = slice(i * C_CORE, (i + 1) * C_CORE)
        # pack: xp[p, g*512 + c] = xf[g*128 + p, 512*i + c]
        xp = np.ascontiguousarray(
            xf[:, cols].reshape(GROUPS, 128, C_CORE).transpose(1, 0, 2)
            .reshape(128, XP_COLS)
        )
        # weights d-major per core: [d, j*128+e] = W[4i+j, d, e], cast bf16
        w_t = np.ascontiguousarray(
            weights[i * NB_CORE:(i + 1) * NB_CORE].transpose(1, 0, 2)
            .reshape(BLK, C_CORE)
        ).astype(ml_dtypes.bfloat16)
        in_maps.append(
            {
                "x": xp,
                "weights": w_t,
                "bias": np.ascontiguousarray(bias[cols][None, :]),
                "ident": ident,
                "ident32": ident32,
                "ones": ones,
            }
        )
    res = run_bass_kernel_spmd(
        nc, in_maps, core_ids=list(range(N_CORES)), trace=trace
    )
    out = np.empty((B_FULL, SIZE), dtype=np.float32)
    for i in range(N_CORES):
        cols = slice(i * C_CORE, (i + 1) * C_CORE)
        op = res.results[i]["out"]
        out[:, cols] = (
            op.reshape(128, GROUPS, C_CORE).transpose(1, 0, 2)
            .reshape(B_FULL, C_CORE)
        )
    return out.reshape(orig_shape), res


def kernel(**inputs):
    out, _ = _run(inputs, trace=False)
    return out


# revision 33
# speedup vs baseline: 1.0360x; 1.0360x over previous
"""Block-diagonal MLP kernel for Trainium2 (8 NeuronCores, expert-sharded).

Computes out = blockdiag_matmul(x, weights) + bias where
  x: [4, 2048, 4096] f32, weights: [32, 128, 128] f32, bias: [4096] f32.

Strategy: shard the 32 diagonal blocks across 8 cores (4 blocks = 512
feature columns each); every core sees all 8192 flattened rows of its
512-column slice.  Per-core DMA is 16.78 MB in + 16.78 MB out + 0.2 MB
consts.  Loads and stores co-flow on the two HWDGE rings, which
together sustain ~430 GB/s (the SBUF-AXI fabric ceiling) -- the body
floor is ~79 us, so the schedule's whole job is to avoid solo-load /
solo-store phases that cap at ~240-340 GB/s.

The host packs each core's x shard as [128, 32768] (partition p holds
the rows congruent to p mod 128, 64 row-groups side by side), so DMA
per-partition lines are 8 KiB (2 KiB descriptors measured ~35% slower
per ring).  x streams through a rotating pool of [128, 2048] chunk
buffers (4 groups each): each load waits for the transposes of the
chunk 4 buffers back, so loads self-pace to compute rate instead of
front-loading, and stores (ready from ~14 us) overlap loads for the
whole body.  Loads ride the ACT ring, stores the Sync ring; the last
two out-tiles store per-group alternating across both rings so the
tail drains at both-ring rate.

Per 512-column group: PE transpose-mode matmuls (fp32) put the
contraction dim on partitions; ACT evacuates the transpose to SBUF
casting fp32->bf16 (free cast -- bf16 halves the real matmul cost);
bf16 matmuls against SBUF-resident bf16 weights (host-cast, the same
4 blocks for all 64 groups); DVE evacuates with the bias add fused.
Transposes run two groups ahead of the consuming matmuls.  The bias
[1,512] row is broadcast to 128 partitions once on-chip via a K=1
ones-matmul.  bf16 is only used for matmul operands (fp32 PSUM
accumulation); max rel err vs the fp32 reference ~2e-3 (gate 2e-2).
"""
import numpy as np
import ml_dtypes
from contextlib import ExitStack

import concourse.mybir as mybir
import concourse.tile as tile
from concourse import bacc
from concourse.bass_utils import run_bass_kernel_spmd

F32 = mybir.dt.float32
BF16 = mybir.dt.bfloat16

SIZE = 4096
NB = 32            # number of diagonal blocks
BLK = 128          # block size
N_CORES = 8
NB_CORE = NB // N_CORES        # 4 blocks per core
C_CORE = NB_CORE * BLK         # 512 feature columns per core
B_FULL = 4 * 2048              # 8192 flattened rows (all on every core)
GROUPS = B_FULL // 128         # 64 row-groups of [128, 512]
XP_COLS = GROUPS * C_CORE      # 32768 packed columns
G_PER_CHUNK = 8                # groups per load chunk [128, 4096]
N_CHUNKS = GROUPS // G_PER_CHUNK
G_PER_OUT = 4                  # groups per store tile [128, 2048]
TAIL_GROUPS = 8                # last groups stored per-group on both rings

_NC_CACHE = {}


def _build_nc():
    nc = bacc.Bacc()
    x_d = nc.declare_dram_parameter("x", [128, XP_COLS], F32, isOutput=False)
    w_d = nc.declare_dram_parameter("weights", [BLK, C_CORE], BF16, isOutput=False)
    b_d = nc.declare_dram_parameter("bias", [1, C_CORE], F32, isOutput=False)
    i_d = nc.declare_dram_parameter("ident", [BLK, BLK], BF16, isOutput=False)
    n_d = nc.declare_dram_parameter("ones", [1, BLK], F32, isOutput=False)
    o_d = nc.declare_dram_parameter("out", [128, XP_COLS], F32, isOutput=True)

    with tile.TileContext(nc) as tc, ExitStack() as ctx:
        consts = ctx.enter_context(tc.tile_pool(name="consts", bufs=1))
        x_pool = ctx.enter_context(tc.tile_pool(name="x", bufs=4))
        xt_pool = ctx.enter_context(tc.tile_pool(name="xt", bufs=6))
        out_pool = ctx.enter_context(tc.tile_pool(name="out", bufs=4))
        tp_pool = ctx.enter_context(tc.tile_pool(name="tp", bufs=3, space="PSUM"))
        mp_pool = ctx.enter_context(tc.tile_pool(name="mp", bufs=3, space="PSUM"))
        bp_pool = ctx.enter_context(tc.tile_pool(name="bp", bufs=1, space="PSUM"))

        ident = consts.tile([BLK, BLK], BF16)
        ones = consts.tile([1, BLK], F32)
        w_sb = consts.tile([BLK, C_CORE], BF16)
        b_row = consts.tile([1, C_CORE], F32)
        bias_sb = consts.tile([128, C_CORE], F32)

        # Consts: identity (needed by the first transpose ~10.5 us in)
        # leads the Sync ring; weights/bias lead the ACT ring ahead of
        # the x stream.
        nc.sync.dma_start(out=ident, in_=i_d[:, :])
        nc.sync.dma_start(out=ones, in_=n_d[:, :])
        nc.scalar.dma_start(out=w_sb, in_=w_d[:, :])
        nc.scalar.dma_start(out=b_row, in_=b_d[:, :])

        # Broadcast bias across partitions: [128,512] = ones.T @ b_row.
        bp = bp_pool.tile([128, C_CORE], F32)
        nc.tensor.matmul(bp, ones, b_row, start=True, stop=True)
        nc.vector.tensor_copy(bias_sb, bp)

        x_chunks = [None] * N_CHUNKS

        def emit_load(c):
            # SWDGE (gpsimd) DMA casts f32 DRAM -> bf16 SBUF inline in the
            # DMA engines: halves the SBUF-fabric bytes on the load side
            # and halves the PE transpose cost, for free.
            xc = x_pool.tile([128, G_PER_CHUNK * C_CORE], BF16)
            base = c * G_PER_CHUNK * C_CORE
            if c == 0:
                # split so the first transposes start earlier
                nc.gpsimd.dma_start(out=xc[:, 0:512], in_=x_d[:, 0:512])
                nc.gpsimd.dma_start(out=xc[:, 512:2048], in_=x_d[:, 512:2048])
                nc.gpsimd.dma_start(out=xc[:, 2048:4096], in_=x_d[:, 2048:4096])
            else:
                nc.gpsimd.dma_start(out=xc, in_=x_d[:, base:base + G_PER_CHUNK * C_CORE])
            x_chunks[c] = xc

        for c in range(3):
            emit_load(c)

        def emit_transposes(g):
            tp = tp_pool.tile([128, C_CORE], BF16)
            xc = x_chunks[g // G_PER_CHUNK]
            gb = (g % G_PER_CHUNK) * C_CORE
            for j in range(NB_CORE):
                nc.tensor.matmul(
                    tp[:, j * 128:(j + 1) * 128],
                    xc[:, gb + j * 128:gb + (j + 1) * 128],
                    ident,
                    is_transpose=True,
                    start=(j == 0),
                    stop=(j == NB_CORE - 1),
                )
            xt = xt_pool.tile([128, C_CORE], BF16)
            nc.scalar.copy(xt, tp)   # PSUM f32 -> SBUF bf16
            return xt

        xt_q = [emit_transposes(0), emit_transposes(1)]
        out_tile = None
        for g in range(GROUPS):
            if g % G_PER_OUT == 0:
                out_tile = out_pool.tile([128, G_PER_OUT * C_CORE], F32)
            # prefetch: 3 chunks (24 groups) ahead of the transposes,
            # which themselves run 2 groups ahead of the matmuls here
            if g % G_PER_CHUNK == 0 and (gc := g // G_PER_CHUNK + 3) < N_CHUNKS:
                emit_load(gc)
            xt = xt_q.pop(0)
            if g + 2 < GROUPS:
                xt_q.append(emit_transposes(g + 2))
            mp = mp_pool.tile([128, C_CORE], F32)
            for j in range(NB_CORE):
                nc.tensor.matmul(
                    mp[:, j * 128:(j + 1) * 128],
                    xt[:, j * 128:(j + 1) * 128],
                    w_sb[:, j * 128:(j + 1) * 128],
                    start=(j == 0),
                    stop=(j == NB_CORE - 1),
                )
            gi = (g % G_PER_OUT) * C_CORE
            nc.vector.tensor_add(out_tile[:, gi:gi + C_CORE], mp, bias_sb)
            # Loads ride the SWDGE queue, so BOTH HWDGE rings carry
            # stores: alternate out-tiles between them (8.4 MB each).
            if g >= GROUPS - TAIL_GROUPS:
                # tail: store per-pair alternating rings so the kernel
                # tail only waits on 512 KiB.
                if g % 2 == 1:
                    eng = nc.sync if g % 4 == 1 else nc.scalar
                    cols = slice((g - 1) * C_CORE, (g + 1) * C_CORE)
                    eng.dma_start(
                        out=o_d[:, cols],
                        in_=out_tile[:, gi - C_CORE:gi + C_CORE],
                    )
            elif g % G_PER_OUT == G_PER_OUT - 1:
                t = g // G_PER_OUT
                eng = nc.sync if t % 2 == 0 else nc.scalar
                cols = slice(t * G_PER_OUT * C_CORE, (t + 1) * G_PER_OUT * C_CORE)
                eng.dma_start(out=o_d[:, cols], in_=out_tile)

    nc.compile()
    return nc


def _get_nc():
    if "nc" not in _NC_CACHE:
        _NC_CACHE["nc"] = _build_nc()
    return _NC_CACHE["nc"]


def _run(inputs, trace=False):
    x = np.asarray(inputs["x"], dtype=np.float32)
    weights = np.asarray(inputs["weights"], dtype=np.float32)
    bias = np.asarray(inputs["bias"], dtype=np.float32)
    orig_shape = x.shape
    xf = x.reshape(B_FULL, SIZE)
    ident = np.eye(BLK, dtype=np.float32).astype(ml_dtypes.bfloat16)
    ones = np.ones((1, BLK), dtype=np.float32)

    nc = _get_nc()
    in_maps = []
    for i in range(N_CORES):
        cols = slice(i * C_CORE, (i + 1) * C_CORE)
        # pack: xp[p, g*512 + c] = xf[g*128 + p, 512*i + c]
        xp = np.ascontiguousarray(
            xf[:, cols].reshape(GROUPS, 128, C_CORE).transpose(1, 0, 2)
            .reshape(128, XP_COLS)
        )
        # weights d-major per core: [d, j*128+e] = W[4i+j, d, e], cast bf16
        w_t = np.ascontiguousarray(
            weights[i * NB_CORE:(i + 1) * NB_CORE].transpose(1, 0, 2)
            .reshape(BLK, C_CORE)
        ).astype(ml_dtypes.bfloat16)
        in_maps.append(
            {
                "x": xp,
                "weights": w_t,
                "bias": np.ascontiguousarray(bias[cols][None, :]),
                "ident": ident,
                "ones": ones,
            }
        )
    res = run_bass_kernel_spmd(
        nc, in_maps, core_ids=list(range(N_CORES)), trace=trace
    )
    out = np.empty((B_FULL, SIZE), dtype=np.float32)
    for i in range(N_CORES):
        cols = slice(i * C_CORE, (i + 1) * C_CORE)
        op = res.results[i]["out"]
        out[:, cols] = (
            op.reshape(128, GROUPS, C_CORE).transpose(1, 0, 2)
            .reshape(B_FULL, C_CORE)
        )
    return out.reshape(orig_shape), res


def kernel(**inputs):
    out, _ = _run(inputs, trace=False)
    return out


# revision 35
# speedup vs baseline: 1.0666x; 1.0296x over previous
"""Block-diagonal MLP kernel for Trainium2 (8 NeuronCores, expert-sharded).

Computes out = blockdiag_matmul(x, weights) + bias where
  x: [4, 2048, 4096] f32, weights: [32, 128, 128] f32, bias: [4096] f32.

Strategy: shard the 32 diagonal blocks across 8 cores (4 blocks = 512
feature columns each); every core sees all 8192 flattened rows of its
512-column slice.  Per-core DMA is 16.78 MB in + 16.78 MB out + 0.2 MB
consts.  Measured per-core HBM read+write tops out at ~430 GB/s, so
the body floor is ~78 us; the schedule's whole job is to keep loads
and stores co-flowing at that rate with no solo phases (a single HWDGE
ring caps at ~240-340 GB/s).

The host packs each core's x shard as [128, 32768] (partition p holds
the rows congruent to p mod 128, 64 row-groups side by side), so DMA
per-partition lines are 8 KiB (2 KiB descriptor lines measured ~35%
slower per ring).  x streams through a rotating pool of [128, 4096]
chunk buffers (8 groups each) on the SWDGE (gpsimd) queue, which casts
f32 DRAM -> bf16 SBUF inline; that frees BOTH HWDGE rings for stores,
which alternate per out-tile (8.4 MB each ring).  Loads self-pace to
compute rate via chunk-buffer reuse, and stores (ready from ~14 us)
overlap loads for the whole body.  The last two out-tiles store
per-pair alternating across both rings so the tail only waits on
512 KiB.

Per 512-column group: PE transpose-mode matmuls (bf16) put the
contraction dim on partitions; ACT evacuates the transpose to SBUF;
bf16 matmuls against SBUF-resident bf16 weights (host-cast, the same
4 blocks for all 64 groups); DVE evacuates with the bias add fused.
Transposes run two groups ahead of the consuming matmuls.  The bias
[1,512] row is broadcast to 128 partitions once on-chip via a K=1
ones-matmul.  bf16 is only used for matmul operands (fp32 PSUM
accumulation); max rel err vs the fp32 reference ~2e-3 (gate 2e-2).
"""
import numpy as np
import ml_dtypes
from contextlib import ExitStack

import concourse.mybir as mybir
import concourse.tile as tile
from concourse import bacc
from concourse.bass_utils import run_bass_kernel_spmd

F32 = mybir.dt.float32
BF16 = mybir.dt.bfloat16

SIZE = 4096
NB = 32            # number of diagonal blocks
BLK = 128          # block size
N_CORES = 8
NB_CORE = NB // N_CORES        # 4 blocks per core
C_CORE = NB_CORE * BLK         # 512 feature columns per core
B_FULL = 4 * 2048              # 8192 flattened rows (all on every core)
GROUPS = B_FULL // 128         # 64 row-groups of [128, 512]
XP_COLS = GROUPS * C_CORE      # 32768 packed columns
G_PER_CHUNK = 8                # groups per load chunk [128, 4096]
N_CHUNKS = GROUPS // G_PER_CHUNK
G_PER_OUT = 4                  # groups per store tile [128, 2048]
TAIL_GROUPS = 8                # last groups stored per-group on both rings

_NC_CACHE = {}


def _build_nc():
    nc = bacc.Bacc()
    x_d = nc.declare_dram_parameter("x", [128, XP_COLS], F32, isOutput=False)
    w_d = nc.declare_dram_parameter("weights", [BLK, C_CORE], BF16, isOutput=False)
    b_d = nc.declare_dram_parameter("bias", [1, C_CORE], F32, isOutput=False)
    i_d = nc.declare_dram_parameter("ident", [BLK, BLK], BF16, isOutput=False)
    n_d = nc.declare_dram_parameter("ones", [1, BLK], F32, isOutput=False)
    o_d = nc.declare_dram_parameter("out", [128, XP_COLS], F32, isOutput=True)

    with tile.TileContext(nc) as tc, ExitStack() as ctx:
        consts = ctx.enter_context(tc.tile_pool(name="consts", bufs=1))
        x_pool = ctx.enter_context(tc.tile_pool(name="x", bufs=3))
        xt_pool = ctx.enter_context(tc.tile_pool(name="xt", bufs=4))
        out_pool = ctx.enter_context(tc.tile_pool(name="out", bufs=4))
        tp_pool = ctx.enter_context(tc.tile_pool(name="tp", bufs=3, space="PSUM"))
        mp_pool = ctx.enter_context(tc.tile_pool(name="mp", bufs=3, space="PSUM"))
        bp_pool = ctx.enter_context(tc.tile_pool(name="bp", bufs=1, space="PSUM"))

        ident = consts.tile([BLK, BLK], BF16)
        ones = consts.tile([1, BLK], F32)
        w_sb = consts.tile([BLK, C_CORE], BF16)
        b_row = consts.tile([1, C_CORE], F32)
        bias_sb = consts.tile([128, C_CORE], F32)

        # Consts: identity (needed by the first transpose ~10.5 us in)
        # leads the Sync ring; weights/bias lead the ACT ring ahead of
        # the x stream.
        nc.sync.dma_start(out=ident, in_=i_d[:, :])
        nc.sync.dma_start(out=ones, in_=n_d[:, :])
        nc.scalar.dma_start(out=w_sb, in_=w_d[:, :])
        nc.scalar.dma_start(out=b_row, in_=b_d[:, :])

        # Broadcast bias across partitions: [128,512] = ones.T @ b_row.
        bp = bp_pool.tile([128, C_CORE], F32)
        nc.tensor.matmul(bp, ones, b_row, start=True, stop=True)
        nc.vector.tensor_copy(bias_sb, bp)

        x_chunks = [None] * N_CHUNKS

        def emit_load(c):
            # SWDGE (gpsimd) DMA casts f32 DRAM -> bf16 SBUF inline in the
            # DMA engines: halves the SBUF-fabric bytes on the load side
            # and halves the PE transpose cost, for free.
            xc = x_pool.tile([128, G_PER_CHUNK * C_CORE], BF16)
            base = c * G_PER_CHUNK * C_CORE
            if c == 0:
                # split so the first transposes start earlier
                nc.gpsimd.dma_start(out=xc[:, 0:512], in_=x_d[:, 0:512])
                nc.gpsimd.dma_start(out=xc[:, 512:2048], in_=x_d[:, 512:2048])
                nc.gpsimd.dma_start(out=xc[:, 2048:4096], in_=x_d[:, 2048:4096])
            else:
                nc.gpsimd.dma_start(out=xc, in_=x_d[:, base:base + G_PER_CHUNK * C_CORE])
            x_chunks[c] = xc

        for c in range(2):
            emit_load(c)

        def emit_transposes(g):
            tp = tp_pool.tile([128, C_CORE], BF16)
            xc = x_chunks[g // G_PER_CHUNK]
            gb = (g % G_PER_CHUNK) * C_CORE
            for j in range(NB_CORE):
                nc.tensor.matmul(
                    tp[:, j * 128:(j + 1) * 128],
                    xc[:, gb + j * 128:gb + (j + 1) * 128],
                    ident,
                    is_transpose=True,
                    start=(j == 0),
                    stop=(j == NB_CORE - 1),
                )
            xt = xt_pool.tile([128, C_CORE], BF16)
            nc.scalar.copy(xt, tp)   # PSUM f32 -> SBUF bf16
            return xt

        xt_q = [emit_transposes(0), emit_transposes(1)]
        out_tile = None
        for g in range(GROUPS):
            if g % G_PER_OUT == 0:
                out_tile = out_pool.tile([128, G_PER_OUT * C_CORE], F32)
            # prefetch: 2 chunks (16 groups) ahead of the transposes,
            # which themselves run 2 groups ahead of the matmuls here
            if g % G_PER_CHUNK == 0 and (gc := g // G_PER_CHUNK + 2) < N_CHUNKS:
                emit_load(gc)
            xt = xt_q.pop(0)
            if g + 2 < GROUPS:
                xt_q.append(emit_transposes(g + 2))
            mp = mp_pool.tile([128, C_CORE], F32)
            for j in range(NB_CORE):
                nc.tensor.matmul(
                    mp[:, j * 128:(j + 1) * 128],
                    xt[:, j * 128:(j + 1) * 128],
                    w_sb[:, j * 128:(j + 1) * 128],
                    start=(j == 0),
                    stop=(j == NB_CORE - 1),
                )
            gi = (g % G_PER_OUT) * C_CORE
            nc.vector.tensor_add(out_tile[:, gi:gi + C_CORE], mp, bias_sb)
            # Loads ride the SWDGE queue, so BOTH HWDGE rings carry
            # stores: alternate out-tiles between them (8.4 MB each).
            if g >= GROUPS - TAIL_GROUPS:
                # tail: store per-pair alternating rings so the kernel
                # tail only waits on 512 KiB.
                if g % 2 == 1:
                    eng = nc.sync if g % 4 == 1 else nc.scalar
                    cols = slice((g - 1) * C_CORE, (g + 1) * C_CORE)
                    eng.dma_start(
                        out=o_d[:, cols],
                        in_=out_tile[:, gi - C_CORE:gi + C_CORE],
                    )
            elif g % G_PER_OUT == G_PER_OUT - 1:
                t = g // G_PER_OUT
                eng = nc.sync if t % 2 == 0 else nc.scalar
                cols = slice(t * G_PER_OUT * C_CORE, (t + 1) * G_PER_OUT * C_CORE)
                eng.dma_start(out=o_d[:, cols], in_=out_tile)

    nc.compile()
    return nc


def _get_nc():
    if "nc" not in _NC_CACHE:
        _NC_CACHE["nc"] = _build_nc()
    return _NC_CACHE["nc"]


def _run(inputs, trace=False):
    x = np.asarray(inputs["x"], dtype=np.float32)
    weights = np.asarray(inputs["weights"], dtype=np.float32)
    bias = np.asarray(inputs["bias"], dtype=np.float32)
    orig_shape = x.shape
    xf = x.reshape(B_FULL, SIZE)
    ident = np.eye(BLK, dtype=np.float32).astype(ml_dtypes.bfloat16)
    ones = np.ones((1, BLK), dtype=np.float32)

    nc = _get_nc()
    in_maps = []
    for i in range(N_CORES):
        cols = slice(i * C_CORE, (i + 1) * C_CORE)
        # pack: xp[p, g*512 + c] = xf[g*128 + p, 512*i + c]
        xp = np.ascontiguousarray(
            xf[:, cols].reshape(GROUPS, 128, C_CORE).transpose(1, 0, 2)
            .reshape(128, XP_COLS)
        )
        # weights d-major per core: [d, j*128+e] = W[4i+j, d, e], cast bf16
        w_t = np.ascontiguousarray(
            weights[i * NB_CORE:(i + 1) * NB_CORE].transpose(1, 0, 2)
            .reshape(BLK, C_CORE)
        ).astype(ml_dtypes.bfloat16)
        in_maps.append(
            {
                "x": xp,
                "weights": w_t,
                "bias": np.ascontiguousarray(bias[cols][None, :]),
                "ident": ident,
                "ones": ones,
            }
        )
    res = run_bass_kernel_spmd(
        nc, in_maps, core_ids=list(range(N_CORES)), trace=trace
    )
    out = np.empty((B_FULL, SIZE), dtype=np.float32)
    for i in range(N_CORES):
        cols = slice(i * C_CORE, (i + 1) * C_CORE)
        op = res.results[i]["out"]
        out[:, cols] = (
            op.reshape(128, GROUPS, C_CORE).transpose(1, 0, 2)
            .reshape(B_FULL, C_CORE)
        )
    return out.reshape(orig_shape), res


def kernel(**inputs):
    out, _ = _run(inputs, trace=False)
    return out
